# revision 17
# baseline (speedup 1.0000x reference)
"""Cross-attention Trainium2 kernel (nn_CrossAttention_24575802868332).

Sharding: 8 cores; core c handles batch b = c//4 and query rows
r = (c%4)*1024 .. +1024.  Embarrassingly parallel, no collectives.
Host pre-transposes/packs inputs p-major (pure layout work) and casts
to bf16; bias is added on the host (bo is zeros in practice anyway).

Per-core on-device computation (all matmuls bf16, fp32 psum accum):
  P1: q^T = Wq^T @ x^T                       qt_sb [128, 4, 1024]
  P2: per m-block of 512 (8 blocks):
      k^T = Wk^T @ ctx^T block               kt [128, 4, 512]
      v   = ctx^T.T @ Wv block, + ones col   vt [128, 4, 8, 65]
      per (q-chunk, head-pair, m-subtile):
        S^T pair (two K=64 matmuls)        -> psum slab [128, 1024]
        P = exp(0.125 * S^T) on ScalarE    -> sbuf bf16 slab
        O_aug[65,512] += v_aug.T @ P        (row 64 = softmax denom l)
      flush O_aug psums -> acc_o [65, 16, 512] (DVE copy/add)
      (block-1 iteration also kicks the deferred Wo load)
  P3: per qc half (so the qc=0 chain hides under block 7's qc=1 work):
      l rows -> 8 partitions via SBUF DMA, reciprocal, partition-
      broadcast via stride-0 SBUF DMA, normalize+cast to bf16 staging,
      DMA staging into pair-packed ko_sb [128, 8, 512]
  P4: out = (O/l) @ Wo (bf16), DMA out in 4 chunks
"""

import os
import sys

sys.path.insert(0, "/opt/trn_rl_repo")

from contextlib import ExitStack

import numpy as np

import concourse.bass as bass
import concourse.tile as tile
from concourse import bacc, mybir

F32 = mybir.dt.float32
F32R = mybir.dt.float32r
BF16 = mybir.dt.bfloat16
AF = mybir.ActivationFunctionType

# Problem constants (hardcoded per contract)
B, N, M = 2, 4096, 4096
DQ, DC, INNER = 1024, 768, 512
H, D = 8, 64
NCORES = 8
NQ = N * B // NCORES  # 1024 query rows per core
QC = 2  # q chunks of 512
MBLK = 512  # m block size
NBLK = M // MBLK  # 8
HP = H // 2  # 4 head pairs
KQ = DQ // 128  # 8 k-chunks for q proj
KC = DC // 128  # 6 k-chunks for k/v proj
MS = MBLK // 128  # 4 m-subtiles per block


def build_nc():
    nc = bacc.Bacc(
        "TRN2",
        target_bir_lowering=False,
        debug=False,
        enable_asserts=False,
        num_devices=NCORES,
    )
    # all weight/activation inputs are host-repacked p-major so each DMA
    # descriptor covers a full contiguous per-partition run
    xT = nc.dram_tensor("xT", [128, KQ * NQ], BF16, kind="ExternalInput").ap()
    ctxT = nc.dram_tensor(
        "ctxT", [NBLK, 128, KC * MBLK], BF16, kind="ExternalInput"
    ).ap()
    wq = nc.dram_tensor("wq", [128, KQ * INNER], BF16, kind="ExternalInput").ap()
    wk = nc.dram_tensor("wk", [128, KC * INNER], BF16, kind="ExternalInput").ap()
    wv = nc.dram_tensor("wv", [128, KC * INNER], BF16, kind="ExternalInput").ap()
    wo = nc.dram_tensor(
        "wo", [128, (INNER // 128) * DQ], BF16, kind="ExternalInput"
    ).ap()
    ones_bf = nc.dram_tensor("ones_bf", [1, 128], BF16, kind="ExternalInput").ap()
    out = nc.dram_tensor("out", [NQ, DQ], BF16, kind="ExternalOutput").ap()

    with tile.TileContext(nc) as tc:
        _emit(nc, tc, xT, ctxT, wq, wk, wv, wo, ones_bf, out)
    nc.compile()
    return nc


def _emit(nc, tc, xT, ctxT, wq, wk, wv, wo, ones_bf, out):
    with ExitStack() as ctx:
        consts = ctx.enter_context(tc.tile_pool(name="consts", bufs=1))
        wq_sb = consts.tile([128, KQ, INNER], BF16, tag="wq")
        wk_sb = consts.tile([128, KC, INNER], BF16, tag="wk")
        wv_sb = consts.tile([128, KC, INNER], BF16, tag="wv")
        wo_sb = consts.tile([128, INNER // 128, DQ], BF16, tag="wo")
        ones_col = consts.tile([128, MS * H], BF16, tag="ones_col")
        wqr = wq.rearrange("p (k n) -> p k n", k=KQ)
        wkr = wk.rearrange("p (k n) -> p k n", k=KC)
        wvr = wv.rearrange("p (k n) -> p k n", k=KC)

        # persistent accumulators + P3/P4 pools: allocated up-front so they
        # own dedicated SBUF (no anti-deps on the P2 pools' space)
        acc = ctx.enter_context(tc.tile_pool(name="acc", bufs=1))
        # acc_o[d(0:64)+l(64), slot j = qc*8 + hp*2 + parity, q 512]
        acc_o = acc.tile([65, 16, 512], F32, tag="acc_o")
        qt_sb = acc.tile([128, HP, NQ], BF16, tag="qt")  # q^T [inner, q]
        norm = ctx.enter_context(tc.tile_pool(name="norm", bufs=1))
        ko_sb = norm.tile([128, H, 512], BF16, tag="ko")
        recip16 = [
            norm.tile([8, 512], F32R, tag=f"recip16_{q}", name=f"recip16_{q}")
            for q in range(QC)
        ]
        dram = ctx.enter_context(tc.tile_pool(name="dram", bufs=1, space="DRAM"))
        recip_d = dram.tile([16, 512], F32R, tag="recip_d")
        bp_pool = ctx.enter_context(tc.tile_pool(name="bp", bufs=8))
        stage_pool = ctx.enter_context(tc.tile_pool(name="stg", bufs=8))
        out_pool = ctx.enter_context(tc.tile_pool(name="outp", bufs=2))

        # P2 SBUF pools hoisted above P1 so block-0/1 context DMAs can be
        # woven into the P1 DMA stream
        ctx_pool = ctx.enter_context(tc.tile_pool(name="ctx", bufs=3))
        kt_pool = ctx.enter_context(tc.tile_pool(name="kt", bufs=2))
        v_pool = ctx.enter_context(tc.tile_pool(name="v", bufs=2))
        p_pool = ctx.enter_context(tc.tile_pool(name="p", bufs=3))
        ctxTr = ctxT.rearrange("b p (k m) -> b p k m", k=KC)
        cx_tiles = {}

        def issue_cx_dma(blk):
            cx = ctx_pool.tile([128, KC, MBLK], BF16, tag="cx", name=f"cx{blk}")
            for kc in range(KC):
                nc.sync.dma_start(out=cx[:, kc, :], in_=ctxTr[blk, :, kc, :])
            cx_tiles[blk] = cx

        # ---- P1: q^T projection (kc-outer, 8 psum banks) ----
        # DMA emission is interleaved (wq chunk, then xt chunk) so the first
        # matmul's inputs land on the first few queues immediately
        with tc.tile_pool(name="xt", bufs=6) as xt_pool, tc.tile_pool(
            name="qps", bufs=8, space="PSUM"
        ) as qps:
            qt_ps = [
                [
                    qps.tile([128, 512], F32, tag="qps", name=f"qtps_{it}_{qc}")
                    for qc in range(QC)
                ]
                for it in range(HP)
            ]
            xTr = xT.rearrange("p (k q) -> p k q", k=KQ)
            for kc in range(KQ):
                nc.sync.dma_start(out=wq_sb[:, kc, :], in_=wqr[:, kc, :])
                xt = xt_pool.tile([128, NQ], BF16, tag="xt")
                nc.sync.dma_start(out=xt[:, 0:512], in_=xTr[:, kc, 0:512])
                nc.sync.dma_start(out=xt[:, 512:NQ], in_=xTr[:, kc, 512:NQ])
                if kc == 3:
                    # block-0 projections need these shortly after P1 warms up
                    for kc2 in range(0, KC, 2):
                        nc.sync.dma_start(
                            out=wk_sb[:, kc2 : kc2 + 2, :],
                            in_=wkr[:, kc2 : kc2 + 2, :],
                        )
                        nc.sync.dma_start(
                            out=wv_sb[:, kc2 : kc2 + 2, :],
                            in_=wvr[:, kc2 : kc2 + 2, :],
                        )
                    ones_bcast = bass.AP(
                        tensor=ones_bf.tensor, offset=0, ap=[[0, 128], [1, MS * H]]
                    )
                    nc.gpsimd.dma_start(out=ones_col, in_=ones_bcast)
                if kc == 4:
                    issue_cx_dma(0)
                if kc == 6:
                    issue_cx_dma(1)
                for it in range(HP):
                    for qc in range(QC):
                        nc.tensor.matmul(
                            qt_ps[it][qc],
                            wq_sb[:, kc, it * 128 : (it + 1) * 128],
                            xt[:, qc * 512 : (qc + 1) * 512],
                            start=(kc == 0),
                            stop=(kc == KQ - 1),
                        )
            for it in range(HP):
                for qc in range(QC):
                    nc.vector.tensor_copy(
                        qt_sb[:, it, qc * 512 : (qc + 1) * 512], qt_ps[it][qc]
                    )

        # ---- P2: m-block loop ----
        with ExitStack() as actx:
            o_ps = actx.enter_context(
                tc.tile_pool(name="ops", bufs=4, space="PSUM")
            )
            s_ps = actx.enter_context(
                tc.tile_pool(name="sps", bufs=2, space="PSUM")
            )

            def make_block_thunks(blk):
                """Per-block projection work as single-instruction thunks,
                for sprinkling among the previous block's attention slabs."""
                st = {}
                th = []

                def t_dma(blk=blk):
                    st["cx"] = cx_tiles.pop(blk)
                    st["kt"] = kt_pool.tile(
                        [128, HP, MBLK], BF16, tag="kt", name=f"kt{blk}"
                    )
                    vt = v_pool.tile(
                        [128, MS, H, 65], BF16, tag="vt", name=f"vt{blk}"
                    )
                    st["vt"] = vt
                    nc.vector.tensor_copy(
                        vt[:, :, :, 64:65],
                        ones_col[:].rearrange("p (a h o) -> p a h o", a=MS, h=H),
                    )

                th.append(t_dma)
                for it in range(HP):
                    for kc in range(KC):
                        def t_kmm(it=it, kc=kc, blk=blk):
                            if kc == 0:
                                st[f"kp{it}"] = o_ps.tile(
                                    [128, 512], F32, tag="ops",
                                    name=f"kp{blk}_{it}",
                                )
                            nc.tensor.matmul(
                                st[f"kp{it}"],
                                wk_sb[:, kc, it * 128 : (it + 1) * 128],
                                st["cx"][:, kc, :],
                                start=(kc == 0),
                                stop=(kc == KC - 1),
                            )
                        th.append(t_kmm)

                    def t_kev(it=it):
                        nc.vector.tensor_copy(st["kt"][:, it, :], st[f"kp{it}"])

                    th.append(t_kev)
                for ms in range(MS):
                    for kc in range(KC):
                        def t_vmm(ms=ms, kc=kc, blk=blk):
                            if kc == 0:
                                st[f"vp{ms}"] = o_ps.tile(
                                    [128, 512], F32, tag="ops",
                                    name=f"vp{blk}_{ms}",
                                )
                            nc.tensor.matmul(
                                st[f"vp{ms}"],
                                st["cx"][:, kc, ms * 128 : (ms + 1) * 128],
                                wv_sb[:, kc, :],
                                start=(kc == 0),
                                stop=(kc == KC - 1),
                            )
                        th.append(t_vmm)

                    def t_vev(ms=ms):
                        nc.vector.tensor_copy(
                            st["vt"][:, ms, :, 0:64],
                            st[f"vp{ms}"][:].rearrange("p (h d) -> p h d", h=H),
                        )

                    th.append(t_vev)
                return st, th

            # prologue: project block 0 eagerly (cx0/cx1 issued during P1)
            cur_st, th0 = make_block_thunks(0)
            for t in th0:
                t()

            for blk in range(NBLK):
                if blk == 1:
                    # load wo now: queues are past the startup burst, and it
                    # finishes long before P4 needs it
                    wor = wo.rearrange("p (k n) -> p k n", k=INNER // 128)
                    for kc in range(INNER // 128):
                        nc.sync.dma_start(
                            out=wo_sb[:, kc, :], in_=wor[:, kc, :]
                        )
                kt = cur_st["kt"]
                vt = cur_st["vt"]
                if blk + 2 < NBLK:
                    issue_cx_dma(blk + 2)
                if blk + 1 < NBLK:
                    next_st, pend = make_block_thunks(blk + 1)
                else:
                    next_st, pend = None, []
                # pop evenly over the 32 slab iterations (fractional pacing)
                n_slabs = HP * QC * MS
                pend_n = len(pend)
                taken = 0

                slab_i = 0
                for qc in range(QC):
                    for hp in range(HP):
                        ops_e = o_ps.tile(
                            [65, 512], F32, tag="ops", name=f"oe{blk}_{hp}_{qc}"
                        )
                        ops_o = o_ps.tile(
                            [65, 512], F32, tag="ops", name=f"oo{blk}_{hp}_{qc}"
                        )
                        o_emits = []
                        for mt in range(MS):
                            sl = s_ps.tile(
                                [128, 1024], F32, tag="sps",
                                name=f"sl{blk}_{hp}_{qc}_{mt}",
                            )
                            nc.tensor.matmul(
                                sl[:, 0:512],
                                kt[0:64, hp, mt * 128 : (mt + 1) * 128],
                                qt_sb[0:64, hp, qc * 512 : (qc + 1) * 512],
                                start=True,
                                stop=True,
                            )
                            nc.tensor.matmul(
                                sl[:, 512:1024],
                                kt[64:128, hp, mt * 128 : (mt + 1) * 128],
                                qt_sb[64:128, hp, qc * 512 : (qc + 1) * 512],
                                start=True,
                                stop=True,
                            )
                            psl = p_pool.tile(
                                [128, 1024], BF16, tag="p",
                                name=f"psl{blk}_{hp}_{qc}_{mt}",
                            )
                            nc.scalar.activation(psl, sl, AF.Exp, scale=0.125)

                            def o_pair(mt=mt, psl=psl, ops_e=ops_e, ops_o=ops_o):
                                nc.tensor.matmul(
                                    ops_e,
                                    vt[:, mt, 2 * hp, :],
                                    psl[:, 0:512],
                                    start=(mt == 0),
                                    stop=(mt == MS - 1),
                                )
                                nc.tensor.matmul(
                                    ops_o,
                                    vt[:, mt, 2 * hp + 1, :],
                                    psl[:, 512:1024],
                                    start=(mt == 0),
                                    stop=(mt == MS - 1),
                                )

                            o_emits.append(o_pair)
                            # software pipeline: O lags S by one slab
                            if mt >= 1:
                                o_emits.pop(0)()
                            # sprinkle next block's projection work
                            want = min(pend_n, (slab_i + 3) * pend_n // n_slabs)
                            while taken < want and pend:
                                pend.pop(0)()
                                taken += 1
                            slab_i += 1
                        while o_emits:
                            o_emits.pop(0)()
                        # flush to accumulators (qc-major slot order)
                        je = qc * 8 + hp * 2 + 0
                        jo = qc * 8 + hp * 2 + 1
                        if blk == 0:
                            nc.vector.tensor_copy(acc_o[:, je, :], ops_e)
                            nc.vector.tensor_copy(acc_o[:, jo, :], ops_o)
                        else:
                            nc.vector.tensor_add(
                                acc_o[:, je, :], acc_o[:, je, :], ops_e
                            )
                            nc.vector.tensor_add(
                                acc_o[:, jo, :], acc_o[:, jo, :], ops_o
                            )
                        # P3 per qc half: one reciprocal call each (the
                        # call costs ~3.3us regardless of partition count);
                        # qc=0's chain hides under block 7's qc=1 work
                        if blk == NBLK - 1 and hp == HP - 1:
                            _norm_half(
                                nc, qc, acc_o, recip16, recip_d, bp_pool,
                                stage_pool, ko_sb,
                            )
                for t in pend:  # any leftovers
                    t()
                if next_st is not None:
                    cur_st = next_st

        # ---- P4: out projection (bf16) ----
        with ExitStack() as nctx:
            ops2 = nctx.enter_context(
                tc.tile_pool(name="ops2", bufs=4, space="PSUM")
            )
            for qt_i in range(NQ // 128):
                qc = qt_i // 4
                ql = qt_i % 4
                ob = out_pool.tile([128, DQ], BF16, tag="outp")
                for nck in range(DQ // 512):
                    pp = ops2.tile([128, 512], F32, tag="ops2")
                    for hp in range(HP):
                        nc.tensor.matmul(
                            pp,
                            ko_sb[:, hp * 2 + qc, ql * 128 : (ql + 1) * 128],
                            wo_sb[:, hp, nck * 512 : (nck + 1) * 512],
                            start=(hp == 0),
                            stop=(hp == HP - 1),
                        )
                    nc.vector.tensor_copy(ob[:, nck * 512 : (nck + 1) * 512], pp)
                for oq in range(2):
                    nc.sync.dma_start(
                        out=out[
                            qt_i * 128 : (qt_i + 1) * 128,
                            oq * 512 : (oq + 1) * 512,
                        ],
                        in_=ob[:, oq * 512 : (oq + 1) * 512],
                    )


def _norm_half(nc, qch, acc_o, recip16, recip_d, bp_pool, stage_pool, ko_sb):
    """Normalize + repack the 8 accumulator slots of one qc half."""
    j0 = qch * 8
    nc.sync.dma_start(
        out=recip16[qch],
        in_=acc_o[64:65, j0 : j0 + 8, :].bitcast(F32R),
    )
    with nc.allow_low_precision(reason="1/l in fp32r is fine"):
        nc.vector.reciprocal(recip16[qch][:], recip16[qch][:])
    # bounce through DRAM: stride-0 partition broadcast needs a DRAM source
    nc.sync.dma_start(out=recip_d[j0 : j0 + 8, :], in_=recip16[qch][:, :])
    for j in range(j0, j0 + 8):
        bp = bp_pool.tile([64, 512], F32R, tag="bp", name=f"bp{j}")
        src = recip_d[j, :]
        bcast = bass.AP(
            tensor=src.tensor, offset=src.offset, ap=[[0, 64], [1, 512]]
        )
        nc.sync.dma_start(out=bp, in_=bcast)
        stage = stage_pool.tile([64, 512], BF16, tag="stg", name=f"stg{j}")
        nc.vector.tensor_mul(stage, acc_o[0:64, j, :], bp)
        hp = (j - j0) // 2
        par = (j - j0) % 2
        j2 = hp * 2 + qch
        nc.sync.dma_start(
            out=ko_sb[par * 64 : (par + 1) * 64, j2, :], in_=stage
        )


_NC_CACHE = None


def _get_nc():
    global _NC_CACHE
    if _NC_CACHE is None:
        _NC_CACHE = build_nc()
    return _NC_CACHE


def _pmajor(a, k):
    """[k*128, n] -> [128, k*n] so each partition's data is contiguous."""
    n = a.shape[1]
    return np.ascontiguousarray(
        a.reshape(k, 128, n).transpose(1, 0, 2).reshape(128, k * n)
    )


def shard_inputs(x, context, Wq, Wk, Wv, Wo, bo):
    import ml_dtypes

    bf16 = ml_dtypes.bfloat16
    ones_b = np.ones((1, 128), bf16)
    Wq = _pmajor(np.asarray(Wq, np.float32).astype(bf16), KQ)
    Wk = _pmajor(np.asarray(Wk, np.float32).astype(bf16), KC)
    Wv = _pmajor(np.asarray(Wv, np.float32).astype(bf16), KC)
    Wo = _pmajor(np.asarray(Wo, np.float32).astype(bf16), INNER // 128)
    ctxs = []
    for b in range(B):
        ct = context[b].T.astype(bf16)  # [DC, M]
        # [NBLK, 128, KC*MBLK], per-block p-major
        ct = (
            ct.reshape(KC, 128, NBLK, MBLK)
            .transpose(2, 1, 0, 3)
            .reshape(NBLK, 128, KC * MBLK)
        )
        ctxs.append(np.ascontiguousarray(ct))
    maps = []
    for c in range(NCORES):
        b = c // 4
        r0 = (c % 4) * NQ
        maps.append(
            {
                "xT": _pmajor(x[b, r0 : r0 + NQ, :].T.astype(bf16), KQ),
                "ctxT": ctxs[b],
                "wq": Wq,
                "wk": Wk,
                "wv": Wv,
                "wo": Wo,
                "ones_bf": ones_b,
            }
        )
    return maps


def kernel(x, context, Wq, Wk, Wv, Wo, bo):
    from concourse.bass_utils import run_bass_kernel_spmd

    x = np.asarray(x, np.float32)
    context = np.asarray(context, np.float32)
    maps = shard_inputs(x, context, Wq, Wk, Wv, Wo, bo)
    nc = _get_nc()
    trace = os.environ.get("KERNEL_TRACE", "0") == "1"
    res = run_bass_kernel_spmd(
        nc, maps, core_ids=list(range(NCORES)), trace=trace
    )
    full = np.empty((B, N, DQ), np.float32)
    for c in range(NCORES):
        b = c // 4
        r0 = (c % 4) * NQ
        full[b, r0 : r0 + NQ, :] = np.asarray(
            res.results[c]["out"], np.float32
        )
    full += np.asarray(bo, np.float32).reshape(1, 1, DQ)
    if trace:
        kernel.last_exec_time_ns = res.exec_time_ns
    return full


# revision 18
# speedup vs baseline: 1.0371x; 1.0371x over previous
"""Cross-attention Trainium2 kernel (nn_CrossAttention_24575802868332).

Sharding: 8 cores; core c handles batch b = c//4 and query rows
r = (c%4)*1024 .. +1024.  Embarrassingly parallel, no collectives.
Host pre-transposes/packs inputs p-major (pure layout work) and casts
to bf16; bias is added on the host (bo is zeros in practice anyway).

Per-core on-device computation (all matmuls bf16, fp32 psum accum):
  P1: q^T = Wq^T @ x^T                       qt_sb [128, 4, 1024]
  P2: per m-block of 512 (8 blocks):
      k^T = Wk^T @ ctx^T block               kt [128, 4, 512]
      v   = ctx^T.T @ Wv block, + ones col   vt [128, 4, 8, 65]
      per (q-chunk, head-pair, m-subtile):
        S^T pair (two K=64 matmuls)        -> psum slab [128, 1024]
        P = exp(0.125 * S^T) on ScalarE    -> sbuf bf16 slab
        O_aug[65,512] += v_aug.T @ P        (row 64 = softmax denom l)
      flush O_aug psums -> acc_o [65, 16, 512] (DVE copy/add)
      (block-1 iteration also kicks the deferred Wo load)
  P3: per qc half (so the qc=0 chain hides under block 7's qc=1 work):
      l rows -> 8 partitions via SBUF DMA, reciprocal, partition-
      broadcast via stride-0 SBUF DMA, normalize+cast to bf16 staging,
      DMA staging into pair-packed ko_sb [128, 8, 512]
  P4: out = (O/l) @ Wo (bf16), DMA out in 4 chunks
"""

import os
import sys

sys.path.insert(0, "/opt/trn_rl_repo")

from contextlib import ExitStack

import numpy as np

import concourse.bass as bass
import concourse.tile as tile
from concourse import bacc, mybir

F32 = mybir.dt.float32
F32R = mybir.dt.float32r
BF16 = mybir.dt.bfloat16
AF = mybir.ActivationFunctionType

# Problem constants (hardcoded per contract)
B, N, M = 2, 4096, 4096
DQ, DC, INNER = 1024, 768, 512
H, D = 8, 64
NCORES = 8
NQ = N * B // NCORES  # 1024 query rows per core
QC = 2  # q chunks of 512
MBLK = 512  # m block size
NBLK = M // MBLK  # 8
HP = H // 2  # 4 head pairs
KQ = DQ // 128  # 8 k-chunks for q proj
KC = DC // 128  # 6 k-chunks for k/v proj
MS = MBLK // 128  # 4 m-subtiles per block


def build_nc():
    nc = bacc.Bacc(
        "TRN2",
        target_bir_lowering=False,
        debug=False,
        enable_asserts=False,
        num_devices=NCORES,
    )
    # all weight/activation inputs are host-repacked p-major so each DMA
    # descriptor covers a full contiguous per-partition run
    xT = nc.dram_tensor("xT", [128, KQ * NQ], BF16, kind="ExternalInput").ap()
    ctxT = nc.dram_tensor(
        "ctxT", [NBLK, 128, KC * MBLK], BF16, kind="ExternalInput"
    ).ap()
    wq = nc.dram_tensor("wq", [128, KQ * INNER], BF16, kind="ExternalInput").ap()
    wk = nc.dram_tensor("wk", [128, KC * INNER], BF16, kind="ExternalInput").ap()
    wv = nc.dram_tensor("wv", [128, KC * INNER], BF16, kind="ExternalInput").ap()
    wo = nc.dram_tensor(
        "wo", [128, (INNER // 128) * DQ], BF16, kind="ExternalInput"
    ).ap()
    ones_bf = nc.dram_tensor("ones_bf", [1, 128], BF16, kind="ExternalInput").ap()
    out = nc.dram_tensor("out", [NQ, DQ], BF16, kind="ExternalOutput").ap()

    with tile.TileContext(nc) as tc:
        _emit(nc, tc, xT, ctxT, wq, wk, wv, wo, ones_bf, out)
    nc.compile()
    return nc


def _emit(nc, tc, xT, ctxT, wq, wk, wv, wo, ones_bf, out):
    with ExitStack() as ctx:
        consts = ctx.enter_context(tc.tile_pool(name="consts", bufs=1))
        wq_sb = consts.tile([128, KQ, INNER], BF16, tag="wq")
        wk_sb = consts.tile([128, KC, INNER], BF16, tag="wk")
        wv_sb = consts.tile([128, KC, INNER], BF16, tag="wv")
        wo_sb = consts.tile([128, INNER // 128, DQ], BF16, tag="wo")
        ones_col = consts.tile([128, MS * H], BF16, tag="ones_col")
        wqr = wq.rearrange("p (k n) -> p k n", k=KQ)
        wkr = wk.rearrange("p (k n) -> p k n", k=KC)
        wvr = wv.rearrange("p (k n) -> p k n", k=KC)

        # persistent accumulators + P3/P4 pools: allocated up-front so they
        # own dedicated SBUF (no anti-deps on the P2 pools' space)
        acc = ctx.enter_context(tc.tile_pool(name="acc", bufs=1))
        # acc_o[d(0:64)+l(64), slot j = qc*8 + hp*2 + parity, q 512]
        acc_o = acc.tile([65, 16, 512], F32, tag="acc_o")
        qt_sb = acc.tile([128, HP, NQ], BF16, tag="qt")  # q^T [inner, q]
        norm = ctx.enter_context(tc.tile_pool(name="norm", bufs=1))
        ko_sb = norm.tile([128, H, 512], BF16, tag="ko")
        recip16 = [
            norm.tile([8, 512], F32R, tag=f"recip16_{q}", name=f"recip16_{q}")
            for q in range(QC)
        ]
        dram = ctx.enter_context(tc.tile_pool(name="dram", bufs=1, space="DRAM"))
        recip_d = dram.tile([16, 512], F32R, tag="recip_d")
        bp_pool = ctx.enter_context(tc.tile_pool(name="bp", bufs=8))
        stage_pool = ctx.enter_context(tc.tile_pool(name="stg", bufs=8))
        out_pool = ctx.enter_context(tc.tile_pool(name="outp", bufs=2))

        # P2 SBUF pools hoisted above P1 so block-0/1 context DMAs can be
        # woven into the P1 DMA stream
        ctx_pool = ctx.enter_context(tc.tile_pool(name="ctx", bufs=3))
        kt_pool = ctx.enter_context(tc.tile_pool(name="kt", bufs=2))
        v_pool = ctx.enter_context(tc.tile_pool(name="v", bufs=2))
        p_pool = ctx.enter_context(tc.tile_pool(name="p", bufs=3))
        ctxTr = ctxT.rearrange("b p (k m) -> b p k m", k=KC)
        cx_tiles = {}

        def issue_cx_dma(blk):
            cx = ctx_pool.tile([128, KC, MBLK], BF16, tag="cx", name=f"cx{blk}")
            for kc in range(KC):
                nc.sync.dma_start(out=cx[:, kc, :], in_=ctxTr[blk, :, kc, :])
            cx_tiles[blk] = cx

        # ---- P1: q^T projection (kc-outer, 8 psum banks) ----
        # DMA emission is interleaved (wq chunk, then xt chunk) so the first
        # matmul's inputs land on the first few queues immediately
        with tc.tile_pool(name="xt", bufs=6) as xt_pool, tc.tile_pool(
            name="qps", bufs=8, space="PSUM"
        ) as qps:
            qt_ps = [
                [
                    qps.tile([128, 512], F32, tag="qps", name=f"qtps_{it}_{qc}")
                    for qc in range(QC)
                ]
                for it in range(HP)
            ]
            xTr = xT.rearrange("p (k q) -> p k q", k=KQ)
            for kc in range(KQ):
                nc.sync.dma_start(out=wq_sb[:, kc, :], in_=wqr[:, kc, :])
                xt = xt_pool.tile([128, NQ], BF16, tag="xt")
                nc.sync.dma_start(out=xt[:, 0:512], in_=xTr[:, kc, 0:512])
                nc.sync.dma_start(out=xt[:, 512:NQ], in_=xTr[:, kc, 512:NQ])
                if kc == 3:
                    # block-0 projections need these shortly after P1 warms up
                    for kc2 in range(0, KC, 2):
                        nc.sync.dma_start(
                            out=wk_sb[:, kc2 : kc2 + 2, :],
                            in_=wkr[:, kc2 : kc2 + 2, :],
                        )
                        nc.sync.dma_start(
                            out=wv_sb[:, kc2 : kc2 + 2, :],
                            in_=wvr[:, kc2 : kc2 + 2, :],
                        )
                    ones_bcast = bass.AP(
                        tensor=ones_bf.tensor, offset=0, ap=[[0, 128], [1, MS * H]]
                    )
                    nc.gpsimd.dma_start(out=ones_col, in_=ones_bcast)

                for it in range(HP):
                    for qc in range(QC):
                        nc.tensor.matmul(
                            qt_ps[it][qc],
                            wq_sb[:, kc, it * 128 : (it + 1) * 128],
                            xt[:, qc * 512 : (qc + 1) * 512],
                            start=(kc == 0),
                            stop=(kc == KQ - 1),
                        )
            for it in range(HP):
                for qc in range(QC):
                    nc.vector.tensor_copy(
                        qt_sb[:, it, qc * 512 : (qc + 1) * 512], qt_ps[it][qc]
                    )

        # ---- P2: m-block loop ----
        with ExitStack() as actx:
            o_ps = actx.enter_context(
                tc.tile_pool(name="ops", bufs=4, space="PSUM")
            )
            s_ps = actx.enter_context(
                tc.tile_pool(name="sps", bufs=2, space="PSUM")
            )

            def make_block_thunks(blk):
                """Per-block projection work as single-instruction thunks,
                for sprinkling among the previous block's attention slabs."""
                st = {}
                th = []

                def t_dma(blk=blk):
                    st["cx"] = cx_tiles.pop(blk)
                    st["kt"] = kt_pool.tile(
                        [128, HP, MBLK], BF16, tag="kt", name=f"kt{blk}"
                    )
                    vt = v_pool.tile(
                        [128, MS, H, 65], BF16, tag="vt", name=f"vt{blk}"
                    )
                    st["vt"] = vt
                    nc.vector.tensor_copy(
                        vt[:, :, :, 64:65],
                        ones_col[:].rearrange("p (a h o) -> p a h o", a=MS, h=H),
                    )

                th.append(t_dma)
                for it in range(HP):
                    for kc in range(KC):
                        def t_kmm(it=it, kc=kc, blk=blk):
                            if kc == 0:
                                st[f"kp{it}"] = o_ps.tile(
                                    [128, 512], F32, tag="ops",
                                    name=f"kp{blk}_{it}",
                                )
                            nc.tensor.matmul(
                                st[f"kp{it}"],
                                wk_sb[:, kc, it * 128 : (it + 1) * 128],
                                st["cx"][:, kc, :],
                                start=(kc == 0),
                                stop=(kc == KC - 1),
                            )
                        th.append(t_kmm)

                    def t_kev(it=it):
                        nc.vector.tensor_copy(st["kt"][:, it, :], st[f"kp{it}"])

                    th.append(t_kev)
                for ms in range(MS):
                    for kc in range(KC):
                        def t_vmm(ms=ms, kc=kc, blk=blk):
                            if kc == 0:
                                st[f"vp{ms}"] = o_ps.tile(
                                    [128, 512], F32, tag="ops",
                                    name=f"vp{blk}_{ms}",
                                )
                            nc.tensor.matmul(
                                st[f"vp{ms}"],
                                st["cx"][:, kc, ms * 128 : (ms + 1) * 128],
                                wv_sb[:, kc, :],
                                start=(kc == 0),
                                stop=(kc == KC - 1),
                            )
                        th.append(t_vmm)

                    def t_vev(ms=ms):
                        nc.vector.tensor_copy(
                            st["vt"][:, ms, :, 0:64],
                            st[f"vp{ms}"][:].rearrange("p (h d) -> p h d", h=H),
                        )

                    th.append(t_vev)
                return st, th

            # prologue: load blocks 0+1 context, project block 0 eagerly
            issue_cx_dma(0)
            issue_cx_dma(1)
            cur_st, th0 = make_block_thunks(0)
            for t in th0:
                t()

            for blk in range(NBLK):
                if blk == 1:
                    # load wo now: queues are past the startup burst, and it
                    # finishes long before P4 needs it
                    wor = wo.rearrange("p (k n) -> p k n", k=INNER // 128)
                    for kc in range(INNER // 128):
                        nc.sync.dma_start(
                            out=wo_sb[:, kc, :], in_=wor[:, kc, :]
                        )
                kt = cur_st["kt"]
                vt = cur_st["vt"]
                if blk + 2 < NBLK:
                    issue_cx_dma(blk + 2)
                if blk + 1 < NBLK:
                    next_st, pend = make_block_thunks(blk + 1)
                else:
                    next_st, pend = None, []
                # pop evenly over the 32 slab iterations (fractional pacing)
                n_slabs = HP * QC * MS
                pend_n = len(pend)
                taken = 0

                slab_i = 0
                for qc in range(QC):
                    for hp in range(HP):
                        ops_e = o_ps.tile(
                            [65, 512], F32, tag="ops", name=f"oe{blk}_{hp}_{qc}"
                        )
                        ops_o = o_ps.tile(
                            [65, 512], F32, tag="ops", name=f"oo{blk}_{hp}_{qc}"
                        )
                        o_emits = []
                        for mt in range(MS):
                            sl = s_ps.tile(
                                [128, 1024], F32, tag="sps",
                                name=f"sl{blk}_{hp}_{qc}_{mt}",
                            )
                            nc.tensor.matmul(
                                sl[:, 0:512],
                                kt[0:64, hp, mt * 128 : (mt + 1) * 128],
                                qt_sb[0:64, hp, qc * 512 : (qc + 1) * 512],
                                start=True,
                                stop=True,
                            )
                            nc.tensor.matmul(
                                sl[:, 512:1024],
                                kt[64:128, hp, mt * 128 : (mt + 1) * 128],
                                qt_sb[64:128, hp, qc * 512 : (qc + 1) * 512],
                                start=True,
                                stop=True,
                            )
                            psl = p_pool.tile(
                                [128, 1024], BF16, tag="p",
                                name=f"psl{blk}_{hp}_{qc}_{mt}",
                            )
                            nc.scalar.activation(psl, sl, AF.Exp, scale=0.125)

                            def o_pair(mt=mt, psl=psl, ops_e=ops_e, ops_o=ops_o):
                                nc.tensor.matmul(
                                    ops_e,
                                    vt[:, mt, 2 * hp, :],
                                    psl[:, 0:512],
                                    start=(mt == 0),
                                    stop=(mt == MS - 1),
                                )
                                nc.tensor.matmul(
                                    ops_o,
                                    vt[:, mt, 2 * hp + 1, :],
                                    psl[:, 512:1024],
                                    start=(mt == 0),
                                    stop=(mt == MS - 1),
                                )

                            o_emits.append(o_pair)
                            # software pipeline: O lags S by one slab
                            if mt >= 1:
                                o_emits.pop(0)()
                            # sprinkle next block's projection work
                            want = min(pend_n, (slab_i + 3) * pend_n // n_slabs)
                            while taken < want and pend:
                                pend.pop(0)()
                                taken += 1
                            slab_i += 1
                        while o_emits:
                            o_emits.pop(0)()
                        # flush to accumulators (qc-major slot order)
                        je = qc * 8 + hp * 2 + 0
                        jo = qc * 8 + hp * 2 + 1
                        if blk == 0:
                            nc.vector.tensor_copy(acc_o[:, je, :], ops_e)
                            nc.vector.tensor_copy(acc_o[:, jo, :], ops_o)
                        else:
                            nc.vector.tensor_add(
                                acc_o[:, je, :], acc_o[:, je, :], ops_e
                            )
                            nc.vector.tensor_add(
                                acc_o[:, jo, :], acc_o[:, jo, :], ops_o
                            )
                        # P3 per qc half: one reciprocal call each (the
                        # call costs ~3.3us regardless of partition count);
                        # qc=0's chain hides under block 7's qc=1 work
                        if blk == NBLK - 1 and hp == HP - 1:
                            _norm_half(
                                nc, qc, acc_o, recip16, recip_d, bp_pool,
                                stage_pool, ko_sb,
                            )
                for t in pend:  # any leftovers
                    t()
                if next_st is not None:
                    cur_st = next_st

        # ---- P4: out projection (bf16) ----
        with ExitStack() as nctx:
            ops2 = nctx.enter_context(
                tc.tile_pool(name="ops2", bufs=4, space="PSUM")
            )
            for qt_i in range(NQ // 128):
                qc = qt_i // 4
                ql = qt_i % 4
                ob = out_pool.tile([128, DQ], BF16, tag="outp")
                for nck in range(DQ // 512):
                    pp = ops2.tile([128, 512], F32, tag="ops2")
                    for hp in range(HP):
                        nc.tensor.matmul(
                            pp,
                            ko_sb[:, hp * 2 + qc, ql * 128 : (ql + 1) * 128],
                            wo_sb[:, hp, nck * 512 : (nck + 1) * 512],
                            start=(hp == 0),
                            stop=(hp == HP - 1),
                        )
                    nc.vector.tensor_copy(ob[:, nck * 512 : (nck + 1) * 512], pp)
                for oq in range(2):
                    nc.sync.dma_start(
                        out=out[
                            qt_i * 128 : (qt_i + 1) * 128,
                            oq * 512 : (oq + 1) * 512,
                        ],
                        in_=ob[:, oq * 512 : (oq + 1) * 512],
                    )


def _norm_half(nc, qch, acc_o, recip16, recip_d, bp_pool, stage_pool, ko_sb):
    """Normalize + repack the 8 accumulator slots of one qc half."""
    j0 = qch * 8
    nc.sync.dma_start(
        out=recip16[qch],
        in_=acc_o[64:65, j0 : j0 + 8, :].bitcast(F32R),
    )
    with nc.allow_low_precision(reason="1/l in fp32r is fine"):
        nc.vector.reciprocal(recip16[qch][:], recip16[qch][:])
    # bounce through DRAM: stride-0 partition broadcast needs a DRAM source
    nc.sync.dma_start(out=recip_d[j0 : j0 + 8, :], in_=recip16[qch][:, :])
    for j in range(j0, j0 + 8):
        bp = bp_pool.tile([64, 512], F32R, tag="bp", name=f"bp{j}")
        src = recip_d[j, :]
        bcast = bass.AP(
            tensor=src.tensor, offset=src.offset, ap=[[0, 64], [1, 512]]
        )
        nc.sync.dma_start(out=bp, in_=bcast)
        stage = stage_pool.tile([64, 512], BF16, tag="stg", name=f"stg{j}")
        nc.vector.tensor_mul(stage, acc_o[0:64, j, :], bp)
        hp = (j - j0) // 2
        par = (j - j0) % 2
        j2 = hp * 2 + qch
        nc.sync.dma_start(
            out=ko_sb[par * 64 : (par + 1) * 64, j2, :], in_=stage
        )


_NC_CACHE = None


def _get_nc():
    global _NC_CACHE
    if _NC_CACHE is None:
        _NC_CACHE = build_nc()
    return _NC_CACHE


def _pmajor(a, k):
    """[k*128, n] -> [128, k*n] so each partition's data is contiguous."""
    n = a.shape[1]
    return np.ascontiguousarray(
        a.reshape(k, 128, n).transpose(1, 0, 2).reshape(128, k * n)
    )


def shard_inputs(x, context, Wq, Wk, Wv, Wo, bo):
    import ml_dtypes

    bf16 = ml_dtypes.bfloat16
    ones_b = np.ones((1, 128), bf16)
    Wq = _pmajor(np.asarray(Wq, np.float32).astype(bf16), KQ)
    Wk = _pmajor(np.asarray(Wk, np.float32).astype(bf16), KC)
    Wv = _pmajor(np.asarray(Wv, np.float32).astype(bf16), KC)
    Wo = _pmajor(np.asarray(Wo, np.float32).astype(bf16), INNER // 128)
    ctxs = []
    for b in range(B):
        ct = context[b].T.astype(bf16)  # [DC, M]
        # [NBLK, 128, KC*MBLK], per-block p-major
        ct = (
            ct.reshape(KC, 128, NBLK, MBLK)
            .transpose(2, 1, 0, 3)
            .reshape(NBLK, 128, KC * MBLK)
        )
        ctxs.append(np.ascontiguousarray(ct))
    maps = []
    for c in range(NCORES):
        b = c // 4
        r0 = (c % 4) * NQ
        maps.append(
            {
                "xT": _pmajor(x[b, r0 : r0 + NQ, :].T.astype(bf16), KQ),
                "ctxT": ctxs[b],
                "wq": Wq,
                "wk": Wk,
                "wv": Wv,
                "wo": Wo,
                "ones_bf": ones_b,
            }
        )
    return maps


def kernel(x, context, Wq, Wk, Wv, Wo, bo):
    from concourse.bass_utils import run_bass_kernel_spmd

    x = np.asarray(x, np.float32)
    context = np.asarray(context, np.float32)
    maps = shard_inputs(x, context, Wq, Wk, Wv, Wo, bo)
    nc = _get_nc()
    trace = os.environ.get("KERNEL_TRACE", "0") == "1"
    res = run_bass_kernel_spmd(
        nc, maps, core_ids=list(range(NCORES)), trace=trace
    )
    full = np.empty((B, N, DQ), np.float32)
    for c in range(NCORES):
        b = c // 4
        r0 = (c % 4) * NQ
        full[b, r0 : r0 + NQ, :] = np.asarray(
            res.results[c]["out"], np.float32
        )
    full += np.asarray(bo, np.float32).reshape(1, 1, DQ)
    if trace:
        kernel.last_exec_time_ns = res.exec_time_ns
    return full
